# revision 36
# baseline (speedup 1.0000x reference)
"""GNN message-passing cell (3-step, 6 SpMMs) on 8 Trainium2 NeuronCores.

Strategy: 1D dest-node sharding. Each core owns 6272 rows (49 tiles of 128).
Per SpMM, edges are grouped by (dest core, source arrival-phase, dest tile);
neighbor features are fetched with dma_gather from an HBM-replicated state
table (built via chunked AllGather), scaled+segment-summed with a one-hot
matmul on the PE, accumulated in an SBUF accumulator, then LayerNorm+GELU.
"""
import os
import sys

sys.path.insert(0, "/opt/trn_rl_repo")

import numpy as np

# ---------------- problem constants (hardcoded; must match reference) -------
N_STEP = 3
N_NODES = 50000
N_ADJ = 6
NNZ = 800000
DIN = 256
D = 128
LN_EPS = 1e-5

NCORES = 8
P = 128
TPC = 49                  # dest tiles per core
RPC = TPC * P             # 6272 rows per core
NPAD = NCORES * RPC       # 50176 padded rows
C0_T, C1_T = 25, 24       # shard-chunk split in tiles (for 2-chunk AllGather)
C0, C1 = C0_T * P, C1_T * P          # 3200 / 3072 rows per core per chunk
T0, T1 = NCORES * C0, NCORES * C1    # table chunk sizes: 25600 / 24576
WCH = int(os.environ.get("KERNEL_WCH", "8"))
                          # gather-window size in 128-edge chunks (1024 idxs:
                          # the SWDGE per-queue descriptor ring is 1024 deep
                          # (ucode-fixed); a larger single dma_gather wedges
                          # the device even with dynamic_dma_scratch_size up)
NSWQ = 4                  # SWDGE queues to rotate gathers over

DT_BF16 = os.environ.get("KERNEL_BF16", "1") == "1"
DBG_STEPS = int(os.environ.get("KERNEL_STEPS", str(N_STEP)))  # debug bisection
DBG_MAXWIN = int(os.environ.get("KERNEL_MAXWIN", "0"))  # 0 = no limit

LAST_RESULTS = {}         # test.py introspection (exec_time etc.)


# ---------------- host-side edge preprocessing ------------------------------
def _prep_spmm(rows, cols, vals):
    """Partition/sort/pad one adjacency's edges.

    Returns per-phase dicts with padded streams (tidx int16, dloc, val) laid
    out per core, plus the shared per-tile chunk counts K[p][t].
    """
    rows = rows.astype(np.int64)
    cols = cols.astype(np.int64)
    dc = rows // RPC
    lr = rows % RPC
    t = lr // P
    dloc = lr % P
    cs = cols // RPC
    ls = cols % RPC
    ph = (ls >= C0).astype(np.int64)
    tidx = np.where(ph == 0, cs * C0 + ls, cs * C1 + (ls - C0))

    key = (dc * 2 + ph) * TPC + t
    # secondary sort by source index inside each group: gathered rows then
    # arrive mostly ascending -> HBM page locality for the 512B random reads
    order = np.argsort(key * (1 << 16) + tidx, kind="stable")
    counts = np.bincount(key, minlength=NCORES * 2 * TPC).reshape(NCORES, 2, TPC)
    K = np.ceil(counts / P).astype(np.int64).max(axis=0)  # [2, TPC]
    K = np.maximum(K, 1)

    src_off = np.zeros(NCORES * 2 * TPC + 1, np.int64)
    np.cumsum(counts.reshape(-1), out=src_off[1:])
    # stream offsets per phase (tile groups packed in order)
    toff = [np.concatenate([[0], np.cumsum(K[p] * P)]) for p in range(2)]
    L = [int(toff[p][-1]) for p in range(2)]

    tidx_s = tidx[order]
    dloc_s = dloc[order]
    val_s = vals[order]

    out = []
    for p in range(2):
        ti = np.zeros((NCORES, L[p]), np.int16)
        dl = np.zeros((NCORES, L[p]), np.float32)
        vl = np.zeros((NCORES, L[p]), np.float32)
        for c in range(NCORES):
            for tt in range(TPC):
                g = (c * 2 + p) * TPC + tt
                a, b = src_off[g], src_off[g + 1]
                n = b - a
                if n == 0:
                    continue
                o = toff[p][tt]
                ti[c, o:o + n] = tidx_s[a:b]
                dl[c, o:o + n] = dloc_s[a:b]
                vl[c, o:o + n] = val_s[a:b]
        out.append(dict(tidx=ti, dloc=dl, val=vl, L=L[p], K=K[p]))
    return out


def _groups(K):
    """Chunk-range [ga, gb) per dest tile from per-tile chunk counts."""
    g0 = np.concatenate([[0], np.cumsum(K)]).astype(np.int64)
    return [(t, int(g0[t]), int(g0[t + 1])) for t in range(TPC)]


def _wrap_idx(a):  # [L] int16 -> [128, L/16]
    return np.tile(a.reshape(-1, 16).T, (NCORES, 1)).astype(np.int16)


def _wrap_pe(a, np_dt):  # [L] -> [128, L/128] (edge e -> partition e%128)
    return np.ascontiguousarray(a.reshape(-1, P).T.astype(np_dt))


# ---------------- bass program ----------------------------------------------
def _build(meta):
    import concourse.bacc as bacc
    import concourse.mybir as mybir
    import concourse.tile as tile

    f32 = mybir.dt.float32
    i16 = mybir.dt.int16
    DT = mybir.dt.bfloat16 if DT_BF16 else f32
    Alu = mybir.AluOpType
    Act = mybir.ActivationFunctionType

    nc = bacc.Bacc("TRN2", target_bir_lowering=False, debug=False,
                   num_devices=NCORES, num_swdge_queues=NSWQ,
                   dynamic_dma_scratch_size=int(
                       os.environ.get("KERNEL_DDSS", "16384")))

    xt_d = nc.dram_tensor("xt", [DIN, RPC], DT, kind="ExternalInput")
    w0_d = nc.dram_tensor("w0", [P, D], DT, kind="ExternalInput")
    w1_d = nc.dram_tensor("w1", [P, D], DT, kind="ExternalInput")
    brep_d = nc.dram_tensor("brep", [P, D], f32, kind="ExternalInput")
    grep_d = nc.dram_tensor("grep", [P, D], f32, kind="ExternalInput")
    berep_d = nc.dram_tensor("berep", [P, D], f32, kind="ExternalInput")
    iota_d = nc.dram_tensor("iotar", [P, D], DT, kind="ExternalInput")
    idx_d, dloc_d, val_d = {}, {}, {}
    for (m, p), L in meta["lengths"].items():
        idx_d[(m, p)] = nc.dram_tensor(f"idx_{m}_{p}", [P, L // 16], i16,
                                       kind="ExternalInput")
        dloc_d[(m, p)] = nc.dram_tensor(f"dloc_{m}_{p}", [P, L // P], DT,
                                        kind="ExternalInput")
        val_d[(m, p)] = nc.dram_tensor(f"val_{m}_{p}", [P, L // P], DT,
                                       kind="ExternalInput")
    out_d = nc.dram_tensor("out", [RPC, D], f32, kind="ExternalOutput")

    ts = lambda t: slice(t * D, (t + 1) * D)

    with tile.TileContext(nc) as tc:
        with (
            tc.tile_pool(name="const", bufs=1) as cp,
            tc.tile_pool(name="acc", bufs=1) as ap_,
            tc.tile_pool(name="xp", bufs=3) as xp,
            tc.tile_pool(name="gp", bufs=int(os.environ.get("KERNEL_GPBUFS", "8"))) as gp,
            tc.tile_pool(name="op", bufs=8) as op_,
            tc.tile_pool(name="ip", bufs=2) as ip,
            tc.tile_pool(name="sp", bufs=4) as stp,
            tc.tile_pool(name="ep", bufs=2) as ep,
            tc.tile_pool(name="ps", bufs=8, space="PSUM") as pp,
            tc.tile_pool(name="dr", bufs=1, space="DRAM") as dp,
        ):
            w0_t = cp.tile([P, D], DT, name="w0t")
            nc.sync.dma_start(w0_t[:], w0_d[:])
            w1_t = cp.tile([P, D], DT, name="w1t")
            nc.sync.dma_start(w1_t[:], w1_d[:])
            brep_t = cp.tile([P, D], f32, name="brept")
            nc.sync.dma_start(brep_t[:], brep_d[:])
            grep_t = cp.tile([P, D], f32, name="grept")
            nc.sync.dma_start(grep_t[:], grep_d[:])
            berep_t = cp.tile([P, D], f32, name="berept")
            nc.sync.dma_start(berep_t[:], berep_d[:])
            iota_t = cp.tile([P, D], DT, name="iotat")
            nc.sync.dma_start(iota_t[:], iota_d[:])
            eps_t = cp.tile([P, 1], f32, name="epst")
            nc.vector.memset(eps_t[:], LN_EPS)

            accum = ap_.tile([P, TPC * D], f32, name="accum")

            tables = {}

            def exchange_part(s, part):
                C, T_ = (C0, T0) if part == 0 else (C1, T1)
                lo = 0 if part == 0 else C0_T * D
                hi = C0_T * D if part == 0 else TPC * D
                agi = dp.tile([C, D], DT, name=f"agi{part}_{s}",
                              tag=f"agi{part}_{s}")
                tab = dp.tile([T_, D], DT, name=f"tab{part}_{s}",
                              tag=f"tab{part}_{s}", addr_space="Shared")
                # cast f32 accum -> bf16 on the idle scalar engine, then a
                # cast-free HWDGE copy (gpsimd cast-DMA would contend with
                # gather descriptor generation on SWDGE queue 0)
                stg = ep.tile([P, C0_T * D], DT, tag="stg")
                n = hi - lo
                nc.scalar.activation(out=stg[:, :n], in_=accum[:, lo:hi],
                                     func=Act.Copy, bias=0.0, scale=1.0)
                nc.sync.dma_start(
                    agi[:].rearrange("(t p) f -> p t f", p=P),
                    stg[:, :n].rearrange("p (t f) -> p t f", f=D))
                nc.gpsimd.collective_compute(
                    "AllGather", Alu.bypass,
                    replica_groups=[list(range(NCORES))],
                    ins=[agi[:]], outs=[tab[:]])
                tables.setdefault(s, [None, None])[part] = tab

            def exchange(s):
                exchange_part(s, 0)
                exchange_part(s, 1)

            ln_done = [False] * TPC

            def ln_tile(t):
                ln_done[t] = True
                X = accum[:, ts(t)]
                s1 = stp.tile([P, 1], f32, tag="s1")
                nc.vector.reduce_sum(out=s1[:], in_=X,
                                     axis=mybir.AxisListType.X)
                mean = stp.tile([P, 1], f32, tag="mean")
                nc.vector.tensor_scalar_mul(out=mean[:], in0=s1[:],
                                            scalar1=1.0 / D)
                xc = xp.tile([P, D], f32, tag="xc")
                nc.vector.tensor_scalar(out=xc[:], in0=X, scalar1=mean[:],
                                        scalar2=None, op0=Alu.subtract)
                sq = xp.tile([P, D], f32, tag="sq")
                nc.vector.tensor_tensor(out=sq[:], in0=xc[:], in1=xc[:],
                                        op=Alu.mult)
                v1 = stp.tile([P, 1], f32, tag="v1")
                nc.vector.reduce_sum(out=v1[:], in_=sq[:],
                                     axis=mybir.AxisListType.X)
                sd = stp.tile([P, 1], f32, tag="sd")
                nc.scalar.activation(out=sd[:], in_=v1[:], func=Act.Sqrt,
                                     bias=eps_t[:], scale=1.0 / D)
                rstd = stp.tile([P, 1], f32, tag="rstd")
                nc.vector.reciprocal(out=rstd[:], in_=sd[:])
                y = xp.tile([P, D], f32, tag="y")
                nc.vector.tensor_scalar(out=y[:], in0=xc[:], scalar1=rstd[:],
                                        scalar2=None, op0=Alu.mult)
                nc.vector.tensor_tensor(out=y[:], in0=y[:], in1=grep_t[:],
                                        op=Alu.mult)
                nc.vector.tensor_tensor(out=y[:], in0=y[:], in1=berep_t[:],
                                        op=Alu.add)
                yo = xp.tile([P, D], f32, tag="yo")
                nc.scalar.activation(out=yo[:], in_=y[:], func=Act.Gelu)
                nc.sync.dma_start(out_d[t * P:(t + 1) * P, :], yo[:])

            # ---------------- affine: h0 = x @ W + b ----------------
            for t in range(TPC):
                xt0 = xp.tile([P, P], DT, tag="xt0")
                nc.sync.dma_start(xt0[:], xt_d[0:P, t * P:(t + 1) * P])
                xt1 = xp.tile([P, P], DT, tag="xt1")
                nc.sync.dma_start(xt1[:], xt_d[P:DIN, t * P:(t + 1) * P])
                ps = pp.tile([P, D], mybir.dt.float32, name="psa", tag="ps")
                nc.tensor.matmul(out=ps[:], lhsT=xt0[:], rhs=w0_t[:],
                                 start=True, stop=False)
                nc.tensor.matmul(out=ps[:], lhsT=xt1[:], rhs=w1_t[:],
                                 start=False, stop=True)
                nc.vector.tensor_tensor(out=accum[:, ts(t)], in0=ps[:],
                                        in1=brep_t[:], op=Alu.add)
                if t == C0_T - 1:
                    exchange_part(0, 0)
            exchange_part(0, 1)

            # ---------------- message-passing steps ----------------
            qctr = [0]            # global SWDGE queue rotation

            def run_range(md, sigma, p, streams, first, g_lo, g_hi):
                """Gather+accumulate groups [g_lo, g_hi) of phase p."""
                idx_t, dloc_t, val_t = streams[p]
                tab = tables[sigma][p]
                groups = meta["groups"][(md, p)]
                c_lo = groups[g_lo][1]
                c_hi = groups[g_hi - 1][2]
                gi = g_lo
                ps = None
                c = c_lo
                nwin = 0
                while c < c_hi:
                    if DBG_MAXWIN and nwin >= DBG_MAXWIN:
                        return
                    nwin += 1
                    wlen = min(WCH, c_hi - c)
                    nidx = wlen * P
                    g_t = gp.tile([P, WCH, D], DT, tag="g")
                    nc.gpsimd.dma_gather(
                        g_t[:, :wlen, :], tab[:],
                        idx_t[:, c * 8:(c + wlen) * 8],
                        nidx, nidx, D,
                        queue_num=qctr[0] % NSWQ)
                    qctr[0] += 1
                    oh = op_.tile([P, WCH, D], DT, tag="oh")
                    dloc_b = dloc_t[:, c:c + wlen].rearrange(
                        "p (c o) -> p c o", o=1).to_broadcast([P, wlen, D])
                    val_b = val_t[:, c:c + wlen].rearrange(
                        "p (c o) -> p c o", o=1).to_broadcast([P, wlen, D])
                    iota_b = iota_t[:].rearrange(
                        "p (c b) -> p c b", c=1).to_broadcast([P, wlen, D])
                    nc.vector.tensor_tensor(
                        out=oh[:, :wlen, :], in0=dloc_b, in1=iota_b,
                        op=Alu.is_equal)
                    nc.vector.tensor_tensor(
                        out=oh[:, :wlen, :], in0=oh[:, :wlen, :],
                        in1=val_b, op=Alu.mult)
                    for kk in range(c, c + wlen):
                        t, ga, gb = groups[gi]
                        if kk == ga:
                            ps = pp.tile([P, D], mybir.dt.float32,
                                         name="psm", tag="ps")
                        nc.tensor.matmul(
                            out=ps[:], lhsT=oh[:, kk - c, :],
                            rhs=g_t[:, kk - c, :],
                            start=(kk == ga), stop=(kk == gb - 1))
                        if kk == gb - 1:
                            if first[t]:
                                # PSUM->accum copy on the idle scalar
                                # engine keeps DVE free for one-hots
                                nc.scalar.activation(
                                    out=accum[:, ts(t)], in_=ps[:],
                                    func=Act.Copy, bias=0.0, scale=1.0)
                                first[t] = False
                            else:
                                nc.vector.tensor_tensor(
                                    out=accum[:, ts(t)],
                                    in0=accum[:, ts(t)], in1=ps[:],
                                    op=Alu.add)
                            gi += 1
                    c += wlen

            def load_streams(md):
                streams = []
                for p in range(2):
                    L = meta["lengths"][(md, p)]
                    idx_t = ip.tile([P, L // 16], i16, tag=f"idx{p}")
                    nc.sync.dma_start(idx_t[:], idx_d[(md, p)][:])
                    dloc_t = ip.tile([P, L // P], DT, tag=f"dloc{p}")
                    nc.sync.dma_start(dloc_t[:], dloc_d[(md, p)][:])
                    val_t = ip.tile([P, L // P], DT, tag=f"val{p}")
                    nc.sync.dma_start(val_t[:], val_d[(md, p)][:])
                    streams.append((idx_t, dloc_t, val_t))
                return streams

            for i, spmms in enumerate(meta["steps"][:DBG_STEPS]):
                first = [True] * TPC
                do_exch = i < DBG_STEPS - 1
                for si_, (m, sigma) in enumerate(spmms):
                    last_spmm = si_ == len(spmms) - 1
                    streams = load_streams(m)
                    if last_spmm and do_exch and not DBG_MAXWIN:
                        # split at tile C0_T: chunk-0 of the new state is
                        # final halfway through this spmm -> AllGather it
                        # while the chunk-1 tiles are still processed
                        run_range(m, sigma, 0, streams, first, 0, C0_T)
                        run_range(m, sigma, 1, streams, first, 0, C0_T)
                        exchange_part(i + 1, 0)
                        run_range(m, sigma, 0, streams, first, C0_T, TPC)
                        run_range(m, sigma, 1, streams, first, C0_T, TPC)
                        exchange_part(i + 1, 1)
                    else:
                        run_range(m, sigma, 0, streams, first, 0, TPC)
                        run_range(m, sigma, 1, streams, first, 0, TPC)
                for t in range(TPC):  # only reachable under DBG_MAXWIN
                    if first[t]:
                        assert DBG_MAXWIN
                        nc.vector.memset(accum[:, ts(t)], 0.0)
                if do_exch and DBG_MAXWIN:
                    exchange_part(i + 1, 0)
                    exchange_part(i + 1, 1)

            # ---------------- LayerNorm + GELU (per-tile leftovers) -------
            for t in range(TPC):
                if not ln_done[t]:
                    ln_tile(t)

    nc.compile()
    n_inst = sum(len(b.instructions) for f in nc.m.functions for b in f.blocks)
    print(f"[kernel] instructions: {n_inst}", flush=True)
    return nc


# ---------------- entry point ------------------------------------------------
def kernel(x, adj_rows, adj_cols, adj_vals, idxes_seq, idxes_res, W, b,
           gamma, beta):
    from concourse.bass_utils import run_bass_kernel_spmd

    np_dt = np.dtype("bfloat16") if False else None  # DT cast below via ml_dtypes
    import ml_dtypes
    np_DT = ml_dtypes.bfloat16 if DT_BF16 else np.float32

    x = np.asarray(x, np.float32)
    W = np.asarray(W, np.float32)
    b = np.asarray(b, np.float32)
    gamma = np.asarray(gamma, np.float32)
    beta = np.asarray(beta, np.float32)
    adj_rows = np.asarray(adj_rows)
    adj_cols = np.asarray(adj_cols)
    adj_vals = np.asarray(adj_vals, np.float32)
    idxes_seq = np.asarray(idxes_seq).astype(np.int64)
    idxes_res = np.asarray(idxes_res).astype(np.int64)

    # spmm list: (m, adjacency a, source state sigma), grouped per step with
    # residuals first so fresh-state gathers land last (overlap with AllGather)
    spmm_specs = []           # m -> (a, sigma)
    steps = []
    off = 0
    for i in range(N_STEP):
        lst = []
        for j in range(i):
            m = len(spmm_specs)
            spmm_specs.append((int(idxes_res[off + j]), j))
            lst.append((m, j))
        m = len(spmm_specs)
        spmm_specs.append((int(idxes_seq[i]), i))
        lst.append((m, i))
        off += i
        steps.append(lst)

    # host prep per spmm
    lengths, groups = {}, {}
    per_core_streams = {}     # (m,p) -> dict arrays per core
    for m, (a, sigma) in enumerate(spmm_specs):
        phases = _prep_spmm(adj_rows[a], adj_cols[a], adj_vals[a])
        for p in range(2):
            ph = phases[p]
            lengths[(m, p)] = ph["L"]
            groups[(m, p)] = _groups(ph["K"])
            per_core_streams[(m, p)] = ph

    meta = dict(lengths=lengths, groups=groups, steps=steps)
    nc = _build(meta)

    # per-core inputs
    xpad = np.zeros((NPAD, DIN), np.float32)
    xpad[:N_NODES] = x
    xt_full = np.ascontiguousarray(xpad.T)

    iota_rep = np.tile(np.arange(D, dtype=np.float32), (P, 1))
    in_maps = []
    for c in range(NCORES):
        im = dict(
            xt=np.ascontiguousarray(
                xt_full[:, c * RPC:(c + 1) * RPC]).astype(np_DT),
            w0=W[:P].astype(np_DT),
            w1=W[P:].astype(np_DT),
            brep=np.tile(b, (P, 1)).astype(np.float32),
            grep=np.tile(gamma, (P, 1)).astype(np.float32),
            berep=np.tile(beta, (P, 1)).astype(np.float32),
            iotar=iota_rep.astype(np_DT),
        )
        for (m, p), ph in per_core_streams.items():
            im[f"idx_{m}_{p}"] = _wrap_idx(ph["tidx"][c])
            im[f"dloc_{m}_{p}"] = _wrap_pe(ph["dloc"][c], np_DT)
            im[f"val_{m}_{p}"] = _wrap_pe(ph["val"][c], np_DT)
        in_maps.append(im)

    trace = os.environ.get("KERNEL_TRACE", "0") == "1"
    r = run_bass_kernel_spmd(nc, in_maps, core_ids=list(range(NCORES)),
                             trace=trace)
    LAST_RESULTS["r"] = r

    full = np.concatenate([r.results[c]["out"] for c in range(NCORES)], axis=0)
    return np.ascontiguousarray(full[:N_NODES]).astype(np.float32)



# revision 37
# speedup vs baseline: 1.0501x; 1.0501x over previous
"""GNN message-passing cell (3-step, 6 SpMMs) on 8 Trainium2 NeuronCores.

Strategy: 1D dest-node sharding. Each core owns 6272 rows (49 tiles of 128).
Per SpMM, edges are grouped by (dest core, source arrival-phase, dest tile);
neighbor features are fetched with dma_gather from an HBM-replicated state
table (built via chunked AllGather), scaled+segment-summed with a one-hot
matmul on the PE, accumulated in an SBUF accumulator, then LayerNorm+GELU.
"""
import os
import sys

sys.path.insert(0, "/opt/trn_rl_repo")

import numpy as np

# ---------------- problem constants (hardcoded; must match reference) -------
N_STEP = 3
N_NODES = 50000
N_ADJ = 6
NNZ = 800000
DIN = 256
D = 128
LN_EPS = 1e-5

NCORES = 8
P = 128
TPC = 49                  # dest tiles per core
RPC = TPC * P             # 6272 rows per core
NPAD = NCORES * RPC       # 50176 padded rows
C0_T, C1_T = 25, 24       # shard-chunk split in tiles (for 2-chunk AllGather)
C0, C1 = C0_T * P, C1_T * P          # 3200 / 3072 rows per core per chunk
T0, T1 = NCORES * C0, NCORES * C1    # table chunk sizes: 25600 / 24576
WCH = int(os.environ.get("KERNEL_WCH", "8"))
                          # gather-window size in 128-edge chunks (1024 idxs:
                          # the SWDGE per-queue descriptor ring is 1024 deep
                          # (ucode-fixed); a larger single dma_gather wedges
                          # the device even with dynamic_dma_scratch_size up)
NSWQ = 4                  # SWDGE queues to rotate gathers over

DT_BF16 = os.environ.get("KERNEL_BF16", "1") == "1"
DBG_STEPS = int(os.environ.get("KERNEL_STEPS", str(N_STEP)))  # debug bisection
DBG_MAXWIN = int(os.environ.get("KERNEL_MAXWIN", "0"))  # 0 = no limit

LAST_RESULTS = {}         # test.py introspection (exec_time etc.)


# ---------------- host-side edge preprocessing ------------------------------
def _prep_spmm(rows, cols, vals):
    """Partition/sort/pad one adjacency's edges.

    Returns per-phase dicts with padded streams (tidx int16, dloc, val) laid
    out per core, plus the shared per-tile chunk counts K[p][t].
    """
    rows = rows.astype(np.int64)
    cols = cols.astype(np.int64)
    dc = rows // RPC
    lr = rows % RPC
    t = lr // P
    dloc = lr % P
    cs = cols // RPC
    ls = cols % RPC
    ph = (ls >= C0).astype(np.int64)
    tidx = np.where(ph == 0, cs * C0 + ls, cs * C1 + (ls - C0))

    key = (dc * 2 + ph) * TPC + t
    # secondary sort by source index inside each group: gathered rows then
    # arrive mostly ascending -> HBM page locality for the 512B random reads
    order = np.argsort(key * (1 << 16) + tidx, kind="stable")
    counts = np.bincount(key, minlength=NCORES * 2 * TPC).reshape(NCORES, 2, TPC)
    K = np.ceil(counts / P).astype(np.int64).max(axis=0)  # [2, TPC]
    K = np.maximum(K, 1)

    src_off = np.zeros(NCORES * 2 * TPC + 1, np.int64)
    np.cumsum(counts.reshape(-1), out=src_off[1:])
    # stream offsets per phase (tile groups packed in order)
    toff = [np.concatenate([[0], np.cumsum(K[p] * P)]) for p in range(2)]
    L = [int(toff[p][-1]) for p in range(2)]

    tidx_s = tidx[order]
    dloc_s = dloc[order]
    val_s = vals[order]

    out = []
    for p in range(2):
        ti = np.zeros((NCORES, L[p]), np.int16)
        dl = np.zeros((NCORES, L[p]), np.float32)
        vl = np.zeros((NCORES, L[p]), np.float32)
        for c in range(NCORES):
            for tt in range(TPC):
                g = (c * 2 + p) * TPC + tt
                a, b = src_off[g], src_off[g + 1]
                n = b - a
                if n == 0:
                    continue
                o = toff[p][tt]
                ti[c, o:o + n] = tidx_s[a:b]
                dl[c, o:o + n] = dloc_s[a:b]
                vl[c, o:o + n] = val_s[a:b]
        out.append(dict(tidx=ti, dloc=dl, val=vl, L=L[p], K=K[p]))
    return out


def _groups(K):
    """Chunk-range [ga, gb) per dest tile from per-tile chunk counts."""
    g0 = np.concatenate([[0], np.cumsum(K)]).astype(np.int64)
    return [(t, int(g0[t]), int(g0[t + 1])) for t in range(TPC)]


def _wrap_idx(a):  # [L] int16 -> [128, L/16]
    return np.tile(a.reshape(-1, 16).T, (NCORES, 1)).astype(np.int16)


def _wrap_pe(a, np_dt):  # [L] -> [128, L/128] (edge e -> partition e%128)
    return np.ascontiguousarray(a.reshape(-1, P).T.astype(np_dt))


# ---------------- bass program ----------------------------------------------
def _build(meta):
    import concourse.bacc as bacc
    import concourse.mybir as mybir
    import concourse.tile as tile

    f32 = mybir.dt.float32
    i16 = mybir.dt.int16
    DT = mybir.dt.bfloat16 if DT_BF16 else f32
    Alu = mybir.AluOpType
    Act = mybir.ActivationFunctionType

    nc = bacc.Bacc("TRN2", target_bir_lowering=False, debug=False,
                   num_devices=NCORES, num_swdge_queues=NSWQ,
                   dynamic_dma_scratch_size=int(
                       os.environ.get("KERNEL_DDSS", "16384")))

    xt_d = nc.dram_tensor("xt", [DIN, RPC], DT, kind="ExternalInput")
    w0_d = nc.dram_tensor("w0", [P, D], DT, kind="ExternalInput")
    w1_d = nc.dram_tensor("w1", [P, D], DT, kind="ExternalInput")
    brep_d = nc.dram_tensor("brep", [P, D], f32, kind="ExternalInput")
    grep_d = nc.dram_tensor("grep", [P, D], f32, kind="ExternalInput")
    berep_d = nc.dram_tensor("berep", [P, D], f32, kind="ExternalInput")
    iota_d = nc.dram_tensor("iotar", [P, D], DT, kind="ExternalInput")
    idx_d, dloc_d, val_d = {}, {}, {}
    for (m, p), L in meta["lengths"].items():
        idx_d[(m, p)] = nc.dram_tensor(f"idx_{m}_{p}", [P, L // 16], i16,
                                       kind="ExternalInput")
        dloc_d[(m, p)] = nc.dram_tensor(f"dloc_{m}_{p}", [P, L // P], DT,
                                        kind="ExternalInput")
        val_d[(m, p)] = nc.dram_tensor(f"val_{m}_{p}", [P, L // P], DT,
                                       kind="ExternalInput")
    out_d = nc.dram_tensor("out", [RPC, D], f32, kind="ExternalOutput")

    ts = lambda t: slice(t * D, (t + 1) * D)

    with tile.TileContext(nc) as tc:
        with (
            tc.tile_pool(name="const", bufs=1) as cp,
            tc.tile_pool(name="acc", bufs=1) as ap_,
            tc.tile_pool(name="xp", bufs=3) as xp,
            tc.tile_pool(name="gp", bufs=int(os.environ.get("KERNEL_GPBUFS", "8"))) as gp,
            tc.tile_pool(name="op", bufs=8) as op_,
            tc.tile_pool(name="ip", bufs=2) as ip,
            tc.tile_pool(name="sp", bufs=4) as stp,
            tc.tile_pool(name="ep", bufs=2) as ep,
            tc.tile_pool(name="ps", bufs=8, space="PSUM") as pp,
            tc.tile_pool(name="dr", bufs=1, space="DRAM") as dp,
        ):
            w0_t = cp.tile([P, D], DT, name="w0t")
            nc.sync.dma_start(w0_t[:], w0_d[:])
            w1_t = cp.tile([P, D], DT, name="w1t")
            nc.sync.dma_start(w1_t[:], w1_d[:])
            brep_t = cp.tile([P, D], f32, name="brept")
            nc.sync.dma_start(brep_t[:], brep_d[:])
            grep_t = cp.tile([P, D], f32, name="grept")
            nc.sync.dma_start(grep_t[:], grep_d[:])
            berep_t = cp.tile([P, D], f32, name="berept")
            nc.sync.dma_start(berep_t[:], berep_d[:])
            iota_t = cp.tile([P, D], DT, name="iotat")
            nc.sync.dma_start(iota_t[:], iota_d[:])
            eps_t = cp.tile([P, 1], f32, name="epst")
            nc.vector.memset(eps_t[:], LN_EPS)

            accum = ap_.tile([P, TPC * D], f32, name="accum")

            tables = {}

            def exchange_part(s, part):
                C, T_ = (C0, T0) if part == 0 else (C1, T1)
                lo = 0 if part == 0 else C0_T * D
                hi = C0_T * D if part == 0 else TPC * D
                agi = dp.tile([C, D], DT, name=f"agi{part}_{s}",
                              tag=f"agi{part}_{s}")
                tab = dp.tile([T_, D], DT, name=f"tab{part}_{s}",
                              tag=f"tab{part}_{s}", addr_space="Shared")
                # cast f32 accum -> bf16 on the idle scalar engine, then a
                # cast-free HWDGE copy (gpsimd cast-DMA would contend with
                # gather descriptor generation on SWDGE queue 0)
                stg = ep.tile([P, C0_T * D], DT, tag="stg")
                n = hi - lo
                nc.scalar.activation(out=stg[:, :n], in_=accum[:, lo:hi],
                                     func=Act.Copy, bias=0.0, scale=1.0)
                nc.sync.dma_start(
                    agi[:].rearrange("(t p) f -> p t f", p=P),
                    stg[:, :n].rearrange("p (t f) -> p t f", f=D))
                nc.gpsimd.collective_compute(
                    "AllGather", Alu.bypass,
                    replica_groups=[list(range(NCORES))],
                    ins=[agi[:]], outs=[tab[:]])
                tables.setdefault(s, [None, None])[part] = tab

            def exchange(s):
                exchange_part(s, 0)
                exchange_part(s, 1)

            ln_done = [False] * TPC

            def ln_tile(t):
                ln_done[t] = True
                X = accum[:, ts(t)]
                s1 = stp.tile([P, 1], f32, tag="s1")
                nc.vector.reduce_sum(out=s1[:], in_=X,
                                     axis=mybir.AxisListType.X)
                mean = stp.tile([P, 1], f32, tag="mean")
                nc.vector.tensor_scalar_mul(out=mean[:], in0=s1[:],
                                            scalar1=1.0 / D)
                xc = xp.tile([P, D], f32, tag="xc")
                nc.vector.tensor_scalar(out=xc[:], in0=X, scalar1=mean[:],
                                        scalar2=None, op0=Alu.subtract)
                sq = xp.tile([P, D], f32, tag="sq")
                nc.vector.tensor_tensor(out=sq[:], in0=xc[:], in1=xc[:],
                                        op=Alu.mult)
                v1 = stp.tile([P, 1], f32, tag="v1")
                nc.vector.reduce_sum(out=v1[:], in_=sq[:],
                                     axis=mybir.AxisListType.X)
                sd = stp.tile([P, 1], f32, tag="sd")
                nc.scalar.activation(out=sd[:], in_=v1[:], func=Act.Sqrt,
                                     bias=eps_t[:], scale=1.0 / D)
                rstd = stp.tile([P, 1], f32, tag="rstd")
                nc.vector.reciprocal(out=rstd[:], in_=sd[:])
                y = xp.tile([P, D], f32, tag="y")
                nc.vector.tensor_scalar(out=y[:], in0=xc[:], scalar1=rstd[:],
                                        scalar2=None, op0=Alu.mult)
                nc.vector.tensor_tensor(out=y[:], in0=y[:], in1=grep_t[:],
                                        op=Alu.mult)
                nc.vector.tensor_tensor(out=y[:], in0=y[:], in1=berep_t[:],
                                        op=Alu.add)
                yo = xp.tile([P, D], f32, tag="yo")
                nc.scalar.activation(out=yo[:], in_=y[:], func=Act.Gelu)
                nc.sync.dma_start(out_d[t * P:(t + 1) * P, :], yo[:])

            # ---------------- affine: h0 = x @ W + b ----------------
            for t in range(TPC):
                xt0 = xp.tile([P, P], DT, tag="xt0")
                nc.sync.dma_start(xt0[:], xt_d[0:P, t * P:(t + 1) * P])
                xt1 = xp.tile([P, P], DT, tag="xt1")
                nc.sync.dma_start(xt1[:], xt_d[P:DIN, t * P:(t + 1) * P])
                ps = pp.tile([P, D], mybir.dt.float32, name="psa", tag="ps")
                nc.tensor.matmul(out=ps[:], lhsT=xt0[:], rhs=w0_t[:],
                                 start=True, stop=False)
                nc.tensor.matmul(out=ps[:], lhsT=xt1[:], rhs=w1_t[:],
                                 start=False, stop=True)
                nc.vector.tensor_tensor(out=accum[:, ts(t)], in0=ps[:],
                                        in1=brep_t[:], op=Alu.add)
                if t == C0_T - 1:
                    exchange_part(0, 0)
            exchange_part(0, 1)

            # ---------------- message-passing steps ----------------
            qctr = [0]            # global SWDGE queue rotation

            def run_range(md, sigma, p, streams, first, g_lo, g_hi):
                """Gather+accumulate groups [g_lo, g_hi) of phase p."""
                idx_t, dloc_t, val_t = streams[p]
                tab = tables[sigma][p]
                groups = meta["groups"][(md, p)]
                c_lo = groups[g_lo][1]
                c_hi = groups[g_hi - 1][2]
                gi = g_lo
                ps = None
                c = c_lo
                nwin = 0
                while c < c_hi:
                    if DBG_MAXWIN and nwin >= DBG_MAXWIN:
                        return
                    nwin += 1
                    wlen = min(WCH, c_hi - c)
                    nidx = wlen * P
                    g_t = gp.tile([P, WCH, D], DT, tag="g")
                    nc.gpsimd.dma_gather(
                        g_t[:, :wlen, :], tab[:],
                        idx_t[:, c * 8:(c + wlen) * 8],
                        nidx, nidx, D,
                        queue_num=qctr[0] % NSWQ)
                    qctr[0] += 1
                    oh = op_.tile([P, WCH, D], DT, tag="oh")
                    dloc_b = dloc_t[:, c:c + wlen].rearrange(
                        "p (c o) -> p c o", o=1).to_broadcast([P, wlen, D])
                    val_b = val_t[:, c:c + wlen].rearrange(
                        "p (c o) -> p c o", o=1).to_broadcast([P, wlen, D])
                    iota_b = iota_t[:].rearrange(
                        "p (c b) -> p c b", c=1).to_broadcast([P, wlen, D])
                    nc.vector.tensor_tensor(
                        out=oh[:, :wlen, :], in0=dloc_b, in1=iota_b,
                        op=Alu.is_equal)
                    nc.vector.tensor_tensor(
                        out=oh[:, :wlen, :], in0=oh[:, :wlen, :],
                        in1=val_b, op=Alu.mult)
                    for kk in range(c, c + wlen):
                        t, ga, gb = groups[gi]
                        if kk == ga:
                            ps = pp.tile([P, D], mybir.dt.float32,
                                         name="psm", tag="ps")
                        nc.tensor.matmul(
                            out=ps[:], lhsT=oh[:, kk - c, :],
                            rhs=g_t[:, kk - c, :],
                            start=(kk == ga), stop=(kk == gb - 1))
                        if kk == gb - 1:
                            if first[t]:
                                # PSUM->accum copy on the idle scalar
                                # engine keeps DVE free for one-hots
                                nc.scalar.activation(
                                    out=accum[:, ts(t)], in_=ps[:],
                                    func=Act.Copy, bias=0.0, scale=1.0)
                                first[t] = False
                            else:
                                nc.vector.tensor_tensor(
                                    out=accum[:, ts(t)],
                                    in0=accum[:, ts(t)], in1=ps[:],
                                    op=Alu.add)
                            gi += 1
                    c += wlen

            def load_streams(md):
                streams = []
                for p in range(2):
                    L = meta["lengths"][(md, p)]
                    idx_t = ip.tile([P, L // 16], i16, tag=f"idx{p}")
                    nc.sync.dma_start(idx_t[:], idx_d[(md, p)][:])
                    dloc_t = ip.tile([P, L // P], DT, tag=f"dloc{p}")
                    nc.sync.dma_start(dloc_t[:], dloc_d[(md, p)][:])
                    val_t = ip.tile([P, L // P], DT, tag=f"val{p}")
                    nc.sync.dma_start(val_t[:], val_d[(md, p)][:])
                    streams.append((idx_t, dloc_t, val_t))
                return streams

            for i, spmms in enumerate(meta["steps"][:DBG_STEPS]):
                first = [True] * TPC
                do_exch = i < DBG_STEPS - 1
                for si_, (m, sigma) in enumerate(spmms):
                    last_spmm = si_ == len(spmms) - 1
                    streams = load_streams(m)
                    if last_spmm and do_exch and not DBG_MAXWIN:
                        # chunk-0 of the new state is final once phase 1
                        # clears tile C0_T-1 -> AllGather it while the
                        # remaining tiles' gathers still run
                        run_range(m, sigma, 0, streams, first, 0, TPC)
                        run_range(m, sigma, 1, streams, first, 0, C0_T)
                        exchange_part(i + 1, 0)
                        run_range(m, sigma, 1, streams, first, C0_T, TPC)
                        exchange_part(i + 1, 1)
                    else:
                        run_range(m, sigma, 0, streams, first, 0, TPC)
                        run_range(m, sigma, 1, streams, first, 0, TPC)
                for t in range(TPC):  # only reachable under DBG_MAXWIN
                    if first[t]:
                        assert DBG_MAXWIN
                        nc.vector.memset(accum[:, ts(t)], 0.0)
                if do_exch and DBG_MAXWIN:
                    exchange_part(i + 1, 0)
                    exchange_part(i + 1, 1)

            # ---------------- LayerNorm + GELU (per-tile leftovers) -------
            for t in range(TPC):
                if not ln_done[t]:
                    ln_tile(t)

    nc.compile()
    n_inst = sum(len(b.instructions) for f in nc.m.functions for b in f.blocks)
    print(f"[kernel] instructions: {n_inst}", flush=True)
    return nc


# ---------------- entry point ------------------------------------------------
def kernel(x, adj_rows, adj_cols, adj_vals, idxes_seq, idxes_res, W, b,
           gamma, beta):
    from concourse.bass_utils import run_bass_kernel_spmd

    np_dt = np.dtype("bfloat16") if False else None  # DT cast below via ml_dtypes
    import ml_dtypes
    np_DT = ml_dtypes.bfloat16 if DT_BF16 else np.float32

    x = np.asarray(x, np.float32)
    W = np.asarray(W, np.float32)
    b = np.asarray(b, np.float32)
    gamma = np.asarray(gamma, np.float32)
    beta = np.asarray(beta, np.float32)
    adj_rows = np.asarray(adj_rows)
    adj_cols = np.asarray(adj_cols)
    adj_vals = np.asarray(adj_vals, np.float32)
    idxes_seq = np.asarray(idxes_seq).astype(np.int64)
    idxes_res = np.asarray(idxes_res).astype(np.int64)

    # spmm list: (m, adjacency a, source state sigma), grouped per step with
    # residuals first so fresh-state gathers land last (overlap with AllGather)
    spmm_specs = []           # m -> (a, sigma)
    steps = []
    off = 0
    for i in range(N_STEP):
        lst = []
        for j in range(i):
            m = len(spmm_specs)
            spmm_specs.append((int(idxes_res[off + j]), j))
            lst.append((m, j))
        m = len(spmm_specs)
        spmm_specs.append((int(idxes_seq[i]), i))
        lst.append((m, i))
        off += i
        steps.append(lst)

    # host prep per spmm
    lengths, groups = {}, {}
    per_core_streams = {}     # (m,p) -> dict arrays per core
    for m, (a, sigma) in enumerate(spmm_specs):
        phases = _prep_spmm(adj_rows[a], adj_cols[a], adj_vals[a])
        for p in range(2):
            ph = phases[p]
            lengths[(m, p)] = ph["L"]
            groups[(m, p)] = _groups(ph["K"])
            per_core_streams[(m, p)] = ph

    meta = dict(lengths=lengths, groups=groups, steps=steps)
    nc = _build(meta)

    # per-core inputs
    xpad = np.zeros((NPAD, DIN), np.float32)
    xpad[:N_NODES] = x
    xt_full = np.ascontiguousarray(xpad.T)

    iota_rep = np.tile(np.arange(D, dtype=np.float32), (P, 1))
    in_maps = []
    for c in range(NCORES):
        im = dict(
            xt=np.ascontiguousarray(
                xt_full[:, c * RPC:(c + 1) * RPC]).astype(np_DT),
            w0=W[:P].astype(np_DT),
            w1=W[P:].astype(np_DT),
            brep=np.tile(b, (P, 1)).astype(np.float32),
            grep=np.tile(gamma, (P, 1)).astype(np.float32),
            berep=np.tile(beta, (P, 1)).astype(np.float32),
            iotar=iota_rep.astype(np_DT),
        )
        for (m, p), ph in per_core_streams.items():
            im[f"idx_{m}_{p}"] = _wrap_idx(ph["tidx"][c])
            im[f"dloc_{m}_{p}"] = _wrap_pe(ph["dloc"][c], np_DT)
            im[f"val_{m}_{p}"] = _wrap_pe(ph["val"][c], np_DT)
        in_maps.append(im)

    trace = os.environ.get("KERNEL_TRACE", "0") == "1"
    r = run_bass_kernel_spmd(nc, in_maps, core_ids=list(range(NCORES)),
                             trace=trace)
    LAST_RESULTS["r"] = r

    full = np.concatenate([r.results[c]["out"] for c in range(NCORES)], axis=0)
    return np.ascontiguousarray(full[:N_NODES]).astype(np.float32)

